# revision 31
# baseline (speedup 1.0000x reference)
"""NVFP4 linear layer kernel for Trainium2 (8 NeuronCores) — mixed bf16/fp8.

y = x @ dequant(W)^T + bias, W is FP4(E2M1) with E4M3 group scales and a
global fp32 scale.

Strategy (column-parallel, out-features sharded 8 ways, O_C=512/core):
  - Host: dequantize W to f32 (numpy; bf16-exact since W has a 5-bit
    significand). Contraction split: 768 of 4096 k-columns go to fp8e4
    (TRN E4M3, max 240) as 6 k-tiles, the rest stay bf16. The fp8
    columns are a stratified 3-of-16 sample from every scale group
    (equalizes per-row fp8 energy) and each packed fp8 k-tile is
    rotated by an orthonormal 128-Hadamard on both x and W (exact:
    (xR)(WR)^T = xW^T) to gaussianize the quantization error and kill
    the max-element error tail. fp8 x pre-scaled by 32, fp8 W by 1/32,
    so both halves accumulate at scale 1 into one PSUM chain.
    Measured on HW vs fp64 reference: rel_norm 1.630e-2, rel_max
    1.657e-2 (budget 2e-2).
  - Device: per 128-token j-tile: 26 bf16 matmuls + 3 fp8 DoubleRow
    matmuls (each covering 2 k-tiles at 2x rate) into one PSUM tile,
    bias epilogue on DVE, y DMA out. W resident in SBUF.
  - PE cost: 64 j-tiles x 29 slots x 512 cols = 950272 cycles = 396us
    at 2.4 GHz (vs 437us all-bf16). Measured 430us (vs 470us bf16
    baseline); the ~34us gap is the HBM-bound warmup head (all 8 cores
    stream W+x at once), PE pstate ramp, and fixed preamble/teardown —
    warmup restructure attempts (fp8-first, staggered chains, dummy
    ramp matmuls, finer W pieces) all measured worse.
"""
import os
import sys

for _p in ("/opt/trn_rl_repo", "/root/.axon_site/_ro/trn_rl_repo"):
    if _p not in sys.path and os.path.isdir(_p):
        sys.path.append(_p)

import numpy as np
import ml_dtypes
import concourse.bass as bass
import concourse.mybir as mybir
import concourse.tile as tile
from concourse.alu_op_type import AluOpType
from concourse.bass_utils import run_bass_kernel_spmd

BF16 = ml_dtypes.bfloat16
E4 = ml_dtypes.float8_e4m3  # TRN fp8e4: IEEE-ish E4M3, max normal 240

# Problem shapes (hardcoded per contract).
B, S, IN, OUT = 4, 2048, 4096, 4096
M = B * S                 # 8192 tokens
NCORES = 8
O_C = OUT // NCORES       # 512 out-features per core
KT = IN // 128            # 32 k-tiles of 128 contraction
KTB = 26                  # bf16 k-tiles
KT8 = KT - KTB            # fp8 k-tiles
G8 = KT8 // 2             # fp8 DoubleRow groups (2 k-tiles each)
GROUP = 16
JT = M // 128             # 64 token tiles of 128
XBF = KTB * 128           # bf16 x free elems per partition per j-tile
X8F = KT8 * 128           # fp8 x free elems per partition per j-tile
WBF = KTB * O_C           # bf16 W free elems per partition
W8F = KT8 * O_C           # fp8 W free elems per partition
X8SCALE = 32.0            # fp8 x pre-scale; fp8 W is divided by the same


def _split_excess_waits(nc, maxw=1):
    """walrus CoreV3 accepts at most one sync-wait per instruction; move
    excess waits onto preceding NoOps on the same engine."""
    for f in nc.m.functions:
        for bb in f.blocks:
            new_insts = []
            for inst in bb.instructions:
                si = inst.sync_info
                if si is not None and si.on_wait and len(si.on_wait) > maxw:
                    waits = list(si.on_wait)
                    excess, keep = waits[:-maxw], waits[-maxw:]
                    for i in range(0, len(excess), maxw):
                        new_insts.append(
                            mybir.InstNoOp(
                                name=nc.get_next_instruction_name(),
                                engine=inst.engine,
                                sync_info=mybir.SyncInfo(
                                    on_wait=excess[i : i + maxw], on_update=[]
                                ),
                                bass_nofuse=True,
                            )
                        )
                    si.on_wait = keep
                new_insts.append(inst)
            bb.instructions[:] = new_insts


def build():
    """Per-core SPMD program.

    Inputs (per core):
      xtb  [JT*128, XBF] bf16: row jj*128+p, col t*128+m = x[jj*128+m, t*128+p]
      xt8  [JT*128, X8F] fp8 : row jj*128+p, col t*128+m =
                               32*x[jj*128+m, (KTB+t)*128+p]
      wtb  [128, WBF] bf16: row p, col t*O_C+o = W[o_glob, t*128+p]
      wt8  [128, W8F] fp8 : row p, col t*O_C+o = W[o_glob, (KTB+t)*128+p]/32
      bias [1, O_C] f32
    Output:
      y    [M, O_C] f32
    """
    dt = mybir.dt
    DR = mybir.MatmulPerfMode.DoubleRow
    nc = bass.Bass("TRN2", target_bir_lowering=False, debug=False)
    xtb = nc.dram_tensor("xtb", [JT * 128, XBF], dt.bfloat16, kind="ExternalInput").ap()
    xt8 = nc.dram_tensor(
        "xt8", [JT * 128, G8, 2, 128], dt.float8e4, kind="ExternalInput"
    ).ap()
    wtb = nc.dram_tensor("wtb", [128, WBF], dt.bfloat16, kind="ExternalInput").ap()
    wt8 = nc.dram_tensor(
        "wt8", [128, G8, 2, O_C], dt.float8e4, kind="ExternalInput"
    ).ap()
    bias = nc.dram_tensor("bias", [1, O_C], dt.float32, kind="ExternalInput").ap()
    y = nc.dram_tensor("y", [M, O_C], dt.float32, kind="ExternalOutput").ap()

    with tile.TileContext(nc) as tc:
        with (
            tc.tile_pool(name="persist", bufs=1) as pp,
            tc.tile_pool(name="xtile", bufs=5) as xp,
            tc.tile_pool(name="x8tile", bufs=5) as xp8,
            tc.tile_pool(name="yout", bufs=4) as yp,
            tc.tile_pool(name="psum", bufs=8, space="PSUM") as psp,
        ):
            # ---- persistent tiles: W (resident) + bias ----
            w_b = pp.tile([128, WBF], dt.bfloat16, tag="wtb")
            w_8 = pp.tile([128, G8, 2, O_C], dt.float8e4, tag="wt8")
            # Step order per chain: bf16 k-tiles first (steps 0..KTB-1),
            # then the fp8 DoubleRow groups (steps KTB..NSTEP-1). W is
            # streamed in graded pieces over the two otherwise-idle
            # queues: tiny first pieces so the first matmuls start ASAP.
            engs = [nc.gpsimd, nc.scalar]
            W_PIECES = [(0, 2), (2, 2), (4, 4), (8, 4), (12, 4), (16, 4),
                        (20, 4), (24, 2), (KTB, 2), (KTB + 2, 1)]
            NSTEP = KTB + G8

            def w_dma(eng, s0, sn):
                # contiguous in both wtb (bf16 steps) and wt8 (fp8 steps)
                if s0 < KTB:
                    assert s0 + sn <= KTB
                    eng.dma_start(
                        w_b[:, s0 * O_C : (s0 + sn) * O_C],
                        wtb[:, s0 * O_C : (s0 + sn) * O_C],
                    )
                else:
                    g0 = s0 - KTB
                    eng.dma_start(w_8[:, g0 : g0 + sn], wt8[:, g0 : g0 + sn])

            for h, (s0, sn) in enumerate(W_PIECES):
                w_dma(engs[h % 2], s0, sn)
            # bias last: not needed until the first epilogue, and the
            # 128-way broadcast read is slow — must not block W
            bias_t = pp.tile([128, O_C], dt.float32, tag="bias")
            nc.scalar.dma_start(bias_t[:], bias.broadcast_to([128, O_C]))

            def load_x(jj):
                xc = xp.tile([128, XBF], dt.bfloat16, tag="xc")
                nc.sync.dma_start(xc[:], xtb[jj * 128 : (jj + 1) * 128, :])
                x8c = xp8.tile([128, G8, 2, 128], dt.float8e4, tag="x8c")
                nc.gpsimd.dma_start(x8c[:], xt8[jj * 128 : (jj + 1) * 128])
                return xc, x8c

            def mm(ps, xc, x8c, s):
                # step s: 0..KTB-1 bf16 k-tile, KTB.. fp8 DoubleRow group
                if s < KTB:
                    nc.tensor.matmul(
                        ps[:],
                        xc[:, s * 128 : (s + 1) * 128],
                        w_b[:, s * O_C : (s + 1) * O_C],
                        start=(s == 0),
                        stop=False,
                    )
                else:
                    g = s - KTB
                    nc.tensor.matmul(
                        ps[:],
                        x8c[:, g],
                        w_8[:, g],
                        start=False,
                        stop=(g == G8 - 1),
                        perf_mode=DR,
                    )

            def epilogue(jj, ps):
                yc = yp.tile([128, O_C], dt.float32, tag="yc")
                nc.vector.tensor_tensor(
                    out=yc[:], in0=ps[:], in1=bias_t[:], op=AluOpType.add
                )
                nc.scalar.dma_start(y[jj * 128 : (jj + 1) * 128, :], yc[:])

            # Warmup: first 3 chains interleaved at W-piece granularity so
            # each arriving W piece feeds 3 chains' worth of matmuls while
            # the rest of W is still streaming in. x for these chains is
            # DMA'd in matching pieces, chain-major per piece.
            WARM = 4
            xws = [
                xp.tile([128, XBF], dt.bfloat16, tag="xc", name=f"xw{c}")
                for c in range(WARM)
            ]
            x8ws = [
                xp8.tile([128, G8, 2, 128], dt.float8e4, tag="x8c", name=f"x8w{c}")
                for c in range(WARM)
            ]
            for s0, sn in W_PIECES:
                for c in range(WARM):
                    if s0 < KTB:
                        nc.sync.dma_start(
                            xws[c][:, s0 * 128 : (s0 + sn) * 128],
                            xtb[c * 128 : (c + 1) * 128, s0 * 128 : (s0 + sn) * 128],
                        )
                    else:
                        g0 = s0 - KTB
                        nc.sync.dma_start(
                            x8ws[c][:, g0 : g0 + sn],
                            xt8[c * 128 : (c + 1) * 128, g0 : g0 + sn],
                        )
            psw = [
                psp.tile([128, O_C], dt.float32, tag="ps", name=f"psw{c}")
                for c in range(WARM)
            ]
            for s0, sn in W_PIECES:
                for c in range(WARM):
                    for s in range(s0, s0 + sn):
                        mm(psw[c], xws[c], x8ws[c], s)
            for c in range(WARM):
                epilogue(c, psw[c])

            for jj in range(WARM, JT):
                xc, x8c = load_x(jj)
                ps = psp.tile([128, O_C], dt.float32, tag="ps")
                for s in range(NSTEP):
                    mm(ps, xc, x8c, s)
                epilogue(jj, ps)

    return nc


def _dequant_np(weight_data, weight_scales, weight_scale_global):
    """numpy port of reference.dequantize_fp4 -> f32 [OUT, IN]."""
    fp4_lut = np.array(
        [0.0, 0.5, 1.0, 1.5, 2.0, 3.0, 4.0, 6.0,
         -0.0, -0.5, -1.0, -1.5, -2.0, -3.0, -4.0, -6.0], dtype=np.float32)
    b = np.arange(256)
    s = np.where((b >> 7) & 1, -1.0, 1.0)
    e = (b >> 3) & 0xF
    m = (b & 7).astype(np.float64)
    normal = s * np.exp2(e - 7.0) * (1.0 + m / 8.0)
    subnormal = s * np.exp2(-6.0) * (m / 8.0)
    e4m3_lut = np.where(e == 0, subnormal, normal).astype(np.float32)

    wd = weight_data.astype(np.int64)
    o_dim = wd.shape[0]
    lo = wd & 0xF
    hi = (wd >> 4) & 0xF
    nib = np.stack([lo, hi], axis=-1).reshape(o_dim, -1)
    w4 = fp4_lut[nib]
    sc = e4m3_lut[weight_scales.astype(np.int64)]
    sc_full = np.repeat(sc, GROUP, axis=1)
    return w4 * sc_full * np.float32(weight_scale_global)


_H128 = None


def _hadamard128():
    global _H128
    if _H128 is None:
        H = np.array([[1.0]])
        while H.shape[0] < 128:
            H = np.block([[H, H], [H, -H]])
        _H128 = H / np.sqrt(128.0)
    return _H128


_SEL = None


def _fp8_selection():
    """Stratified 3-of-16 sample from every scale group (768 columns)."""
    global _SEL
    if _SEL is None:
        rng = np.random.default_rng(2)
        sel = np.zeros(IN, bool)
        for g in range(IN // GROUP):
            sel[rng.choice(GROUP, KT8 * 128 * GROUP // IN, replace=False)
                + g * GROUP] = True
        _SEL = sel
    return _SEL


def marshal(x, weight_data, weight_scales, weight_scale_global, bias,
            n_cores=NCORES):
    """Host-side marshaling: dequantize + dtype split + device layouts."""
    sel = _fp8_selection()
    H = _hadamard128()
    xf = np.ascontiguousarray(x.reshape(M, IN)).astype(np.float64)
    # bf16 part: [M, XBF] -> [JT,128m,KTB,128p] -> [JT,128p,KTB,128m]
    xb = xf[:, ~sel].astype(BF16)
    xb = xb.reshape(JT, 128, KTB, 128).transpose(0, 3, 2, 1)
    xb = np.ascontiguousarray(xb).reshape(JT * 128, XBF)
    # fp8 part: per-k-tile Hadamard rotation, then pre-scaled e4m3
    xq = xf[:, sel]
    for t in range(KT8):
        s = slice(t * 128, (t + 1) * 128)
        xq[:, s] = xq[:, s] @ H
    x8 = np.clip(xq * X8SCALE, -240.0, 240.0).astype(E4)
    x8 = x8.reshape(JT, 128, KT8, 128).transpose(0, 3, 2, 1)
    x8 = np.ascontiguousarray(x8).reshape(JT * 128, G8, 2, 128)

    W = _dequant_np(weight_data, weight_scales, weight_scale_global).astype(
        np.float64
    )
    # bf16 part: [OUT, XBF] -> [128p, KTB, OUT]
    Wb = W[:, ~sel].astype(BF16)
    Wb = Wb.T.reshape(KTB, 128, OUT).transpose(1, 0, 2)
    # fp8 part: same rotation, scaled down to compensate x8's pre-scale
    Wq = W[:, sel]
    for t in range(KT8):
        s = slice(t * 128, (t + 1) * 128)
        Wq[:, s] = Wq[:, s] @ H
    W8 = np.clip(Wq / X8SCALE, -240.0, 240.0).astype(E4)
    W8 = W8.T.reshape(KT8, 128, OUT).transpose(1, 0, 2)

    bias_f = bias.astype(np.float32)
    in_maps = []
    for c in range(n_cores):
        sl = slice(c * O_C, (c + 1) * O_C)
        in_maps.append(
            {
                "xtb": xb,
                "xt8": x8,
                "wtb": np.ascontiguousarray(Wb[:, :, sl]).reshape(128, WBF),
                "wt8": np.ascontiguousarray(W8[:, :, sl]).reshape(128, G8, 2, O_C),
                "bias": np.ascontiguousarray(bias_f[sl].reshape(1, O_C)),
            }
        )
    return in_maps


_NC_CACHE = {}


def run(x, weight_data, weight_scales, weight_scale_global, bias, trace=False):
    if "mix" not in _NC_CACHE:
        nc = build()
        _split_excess_waits(nc)
        _NC_CACHE["mix"] = nc
    nc = _NC_CACHE["mix"]
    in_maps = marshal(
        np.asarray(x), np.asarray(weight_data), np.asarray(weight_scales),
        np.asarray(weight_scale_global), np.asarray(bias),
    )
    res = run_bass_kernel_spmd(nc, in_maps, list(range(NCORES)), trace=trace)
    y = np.concatenate([res.results[c]["y"] for c in range(NCORES)], axis=1)
    return np.ascontiguousarray(y.reshape(B, S, OUT).astype(np.float32)), res


MODE = "mix-fp8x6-had-v2"


def kernel(x, weight_data, weight_scales, weight_scale_global, bias):
    y, _ = run(x, weight_data, weight_scales, weight_scale_global, bias)
    return y


# revision 33
# speedup vs baseline: 1.1914x; 1.1914x over previous
"""NVFP4 linear layer kernel for Trainium2 (8 NeuronCores) — mixed bf16/fp8.

y = x @ dequant(W)^T + bias, W is FP4(E2M1) with E4M3 group scales and a
global fp32 scale.

Strategy (column-parallel, out-features sharded 8 ways, O_C=512/core):
  - Host: dequantize W to f32 (numpy; bf16-exact since W has a 5-bit
    significand). Contraction split: 768 of 4096 k-columns go to fp8e4
    (TRN E4M3, max 240) as 6 k-tiles, the rest stay bf16. The fp8
    columns are a stratified 3-of-16 sample from every scale group
    (equalizes per-row fp8 energy) and each packed fp8 k-tile is
    rotated by an orthonormal 128-Hadamard on both x and W (exact:
    (xR)(WR)^T = xW^T) to gaussianize the quantization error and kill
    the max-element error tail. fp8 x pre-scaled by 32, fp8 W by 1/32,
    so both halves accumulate at scale 1 into one PSUM chain.
    Measured on HW vs fp64 reference: rel_norm 1.630e-2, rel_max
    1.657e-2 (budget 2e-2).
  - Device: per 128-token j-tile: 26 bf16 matmuls + 3 fp8 DoubleRow
    matmuls (each covering 2 k-tiles at 2x rate) into one PSUM tile,
    bias epilogue on DVE, y DMA out. W resident in SBUF.
  - PE cost: 64 j-tiles x 29 slots x 512 cols = 950272 cycles = 396us
    at 2.4 GHz (vs 437us all-bf16). Measured 430us (vs 470us bf16
    baseline); the ~34us gap is the HBM-bound warmup head (all 8 cores
    stream W+x at once), PE pstate ramp, and fixed preamble/teardown —
    warmup restructure attempts (fp8-first, staggered chains, dummy
    ramp matmuls, finer W pieces) all measured worse.
"""
import os
import sys

for _p in ("/opt/trn_rl_repo", "/root/.axon_site/_ro/trn_rl_repo"):
    if _p not in sys.path and os.path.isdir(_p):
        sys.path.append(_p)

import numpy as np
import ml_dtypes
import concourse.bass as bass
import concourse.mybir as mybir
import concourse.tile as tile
from concourse.alu_op_type import AluOpType
from concourse.bass_utils import run_bass_kernel_spmd

BF16 = ml_dtypes.bfloat16
E4 = ml_dtypes.float8_e4m3  # TRN fp8e4: IEEE-ish E4M3, max normal 240

# Problem shapes (hardcoded per contract).
B, S, IN, OUT = 4, 2048, 4096, 4096
M = B * S                 # 8192 tokens
NCORES = 8
O_C = OUT // NCORES       # 512 out-features per core
KT = IN // 128            # 32 k-tiles of 128 contraction
KTB = 26                  # bf16 k-tiles
KT8 = KT - KTB            # fp8 k-tiles
G8 = KT8 // 2             # fp8 DoubleRow groups (2 k-tiles each)
GROUP = 16
JT = M // 128             # 64 token tiles of 128
XBF = KTB * 128           # bf16 x free elems per partition per j-tile
X8F = KT8 * 128           # fp8 x free elems per partition per j-tile
WBF = KTB * O_C           # bf16 W free elems per partition
W8F = KT8 * O_C           # fp8 W free elems per partition
X8SCALE = 32.0            # fp8 x pre-scale; fp8 W is divided by the same


def _split_excess_waits(nc, maxw=1):
    """walrus CoreV3 accepts at most one sync-wait per instruction; move
    excess waits onto preceding NoOps on the same engine."""
    for f in nc.m.functions:
        for bb in f.blocks:
            new_insts = []
            for inst in bb.instructions:
                si = inst.sync_info
                if si is not None and si.on_wait and len(si.on_wait) > maxw:
                    waits = list(si.on_wait)
                    excess, keep = waits[:-maxw], waits[-maxw:]
                    for i in range(0, len(excess), maxw):
                        new_insts.append(
                            mybir.InstNoOp(
                                name=nc.get_next_instruction_name(),
                                engine=inst.engine,
                                sync_info=mybir.SyncInfo(
                                    on_wait=excess[i : i + maxw], on_update=[]
                                ),
                                bass_nofuse=True,
                            )
                        )
                    si.on_wait = keep
                new_insts.append(inst)
            bb.instructions[:] = new_insts


def build():
    """Per-core SPMD program.

    Inputs (per core):
      xtb  [JT*128, XBF] bf16: row jj*128+p, col t*128+m = x[jj*128+m, t*128+p]
      xt8  [JT*128, X8F] fp8 : row jj*128+p, col t*128+m =
                               32*x[jj*128+m, (KTB+t)*128+p]
      wtb  [128, WBF] bf16: row p, col t*O_C+o = W[o_glob, t*128+p]
      wt8  [128, W8F] fp8 : row p, col t*O_C+o = W[o_glob, (KTB+t)*128+p]/32
      bias [1, O_C] f32
    Output:
      y    [M, O_C] f32
    """
    dt = mybir.dt
    DR = mybir.MatmulPerfMode.DoubleRow
    nc = bass.Bass("TRN2", target_bir_lowering=False, debug=False)
    xtb = nc.dram_tensor("xtb", [JT * 128, XBF], dt.bfloat16, kind="ExternalInput").ap()
    xt8 = nc.dram_tensor(
        "xt8", [JT * 128, G8, 2, 128], dt.float8e4, kind="ExternalInput"
    ).ap()
    wtb = nc.dram_tensor("wtb", [128, WBF], dt.bfloat16, kind="ExternalInput").ap()
    wt8 = nc.dram_tensor(
        "wt8", [128, G8, 2, O_C], dt.float8e4, kind="ExternalInput"
    ).ap()
    bias = nc.dram_tensor("bias", [1, O_C], dt.float32, kind="ExternalInput").ap()
    y = nc.dram_tensor("y", [M, O_C], dt.float32, kind="ExternalOutput").ap()

    with tile.TileContext(nc) as tc:
        with (
            tc.tile_pool(name="persist", bufs=1) as pp,
            tc.tile_pool(name="xtile", bufs=5) as xp,
            tc.tile_pool(name="x8tile", bufs=5) as xp8,
            tc.tile_pool(name="yout", bufs=4) as yp,
            tc.tile_pool(name="psum", bufs=8, space="PSUM") as psp,
        ):
            # ---- persistent tiles: W (resident) + bias ----
            w_b = pp.tile([128, WBF], dt.bfloat16, tag="wtb")
            w_8 = pp.tile([128, G8, 2, O_C], dt.float8e4, tag="wt8")
            # Step order per chain: bf16 k-tiles first (steps 0..KTB-1),
            # then the fp8 DoubleRow groups (steps KTB..NSTEP-1). W is
            # streamed in graded pieces over the two otherwise-idle
            # queues: tiny first pieces so the first matmuls start ASAP.
            engs = [nc.gpsimd, nc.scalar]
            W_PIECES = [(0, 2), (2, 2), (4, 4), (8, 4), (12, 4), (16, 4),
                        (20, 4), (24, 2), (KTB, 2), (KTB + 2, 1)]
            NSTEP = KTB + G8

            def w_dma(eng, s0, sn):
                # contiguous in both wtb (bf16 steps) and wt8 (fp8 steps)
                if s0 < KTB:
                    assert s0 + sn <= KTB
                    eng.dma_start(
                        w_b[:, s0 * O_C : (s0 + sn) * O_C],
                        wtb[:, s0 * O_C : (s0 + sn) * O_C],
                    )
                else:
                    g0 = s0 - KTB
                    eng.dma_start(w_8[:, g0 : g0 + sn], wt8[:, g0 : g0 + sn])

            for h, (s0, sn) in enumerate(W_PIECES):
                w_dma(engs[h % 2], s0, sn)
            # bias last: not needed until the first epilogue, and the
            # 128-way broadcast read is slow — must not block W
            bias_t = pp.tile([128, O_C], dt.float32, tag="bias")
            nc.scalar.dma_start(bias_t[:], bias.broadcast_to([128, O_C]))

            def load_x(jj):
                xc = xp.tile([128, XBF], dt.bfloat16, tag="xc")
                nc.sync.dma_start(xc[:], xtb[jj * 128 : (jj + 1) * 128, :])
                x8c = xp8.tile([128, G8, 2, 128], dt.float8e4, tag="x8c")
                nc.gpsimd.dma_start(x8c[:], xt8[jj * 128 : (jj + 1) * 128])
                return xc, x8c

            def mm(ps, xc, x8c, s):
                # step s: 0..KTB-1 bf16 k-tile, KTB.. fp8 DoubleRow group
                if s < KTB:
                    nc.tensor.matmul(
                        ps[:],
                        xc[:, s * 128 : (s + 1) * 128],
                        w_b[:, s * O_C : (s + 1) * O_C],
                        start=(s == 0),
                        stop=False,
                    )
                else:
                    g = s - KTB
                    nc.tensor.matmul(
                        ps[:],
                        x8c[:, g],
                        w_8[:, g],
                        start=False,
                        stop=(g == G8 - 1),
                        perf_mode=DR,
                    )

            def epilogue(jj, ps):
                yc = yp.tile([128, O_C], dt.float32, tag="yc")
                nc.vector.tensor_tensor(
                    out=yc[:], in0=ps[:], in1=bias_t[:], op=AluOpType.add
                )
                nc.scalar.dma_start(y[jj * 128 : (jj + 1) * 128, :], yc[:])

            # Dummy matmuls on memset data: wake the PE and keep its
            # frequency ramp going until the first real matmul's data
            # lands (~12.6us); idle decays the pstate, so they must abut.
            scratch = pp.tile([128, 640], dt.bfloat16, tag="scratch")
            nc.vector.memset(scratch[:], 0.0)
            ps_d = psp.tile([128, O_C], dt.float32, tag="ps", name="ps_dummy")
            for _ in range(7):
                nc.tensor.matmul(
                    ps_d[:], scratch[:, :128], scratch[:, 128:640],
                    start=True, stop=True,
                )

            # Warmup: first 3 chains interleaved at W-piece granularity so
            # each arriving W piece feeds 3 chains' worth of matmuls while
            # the rest of W is still streaming in. x for these chains is
            # DMA'd in matching pieces, chain-major per piece.
            WARM = 3
            xws = [
                xp.tile([128, XBF], dt.bfloat16, tag="xc", name=f"xw{c}")
                for c in range(WARM)
            ]
            x8ws = [
                xp8.tile([128, G8, 2, 128], dt.float8e4, tag="x8c", name=f"x8w{c}")
                for c in range(WARM)
            ]
            for s0, sn in W_PIECES:
                for c in range(WARM):
                    if s0 < KTB:
                        nc.sync.dma_start(
                            xws[c][:, s0 * 128 : (s0 + sn) * 128],
                            xtb[c * 128 : (c + 1) * 128, s0 * 128 : (s0 + sn) * 128],
                        )
                    else:
                        g0 = s0 - KTB
                        nc.sync.dma_start(
                            x8ws[c][:, g0 : g0 + sn],
                            xt8[c * 128 : (c + 1) * 128, g0 : g0 + sn],
                        )
            psw = [
                psp.tile([128, O_C], dt.float32, tag="ps", name=f"psw{c}")
                for c in range(WARM)
            ]
            for s0, sn in W_PIECES:
                for c in range(WARM):
                    for s in range(s0, s0 + sn):
                        mm(psw[c], xws[c], x8ws[c], s)
            for c in range(WARM):
                epilogue(c, psw[c])

            for jj in range(WARM, JT):
                xc, x8c = load_x(jj)
                ps = psp.tile([128, O_C], dt.float32, tag="ps")
                for s in range(NSTEP):
                    mm(ps, xc, x8c, s)
                epilogue(jj, ps)

    return nc


def _dequant_np(weight_data, weight_scales, weight_scale_global):
    """numpy port of reference.dequantize_fp4 -> f32 [OUT, IN]."""
    fp4_lut = np.array(
        [0.0, 0.5, 1.0, 1.5, 2.0, 3.0, 4.0, 6.0,
         -0.0, -0.5, -1.0, -1.5, -2.0, -3.0, -4.0, -6.0], dtype=np.float32)
    b = np.arange(256)
    s = np.where((b >> 7) & 1, -1.0, 1.0)
    e = (b >> 3) & 0xF
    m = (b & 7).astype(np.float64)
    normal = s * np.exp2(e - 7.0) * (1.0 + m / 8.0)
    subnormal = s * np.exp2(-6.0) * (m / 8.0)
    e4m3_lut = np.where(e == 0, subnormal, normal).astype(np.float32)

    wd = weight_data.astype(np.int64)
    o_dim = wd.shape[0]
    lo = wd & 0xF
    hi = (wd >> 4) & 0xF
    nib = np.stack([lo, hi], axis=-1).reshape(o_dim, -1)
    w4 = fp4_lut[nib]
    sc = e4m3_lut[weight_scales.astype(np.int64)]
    sc_full = np.repeat(sc, GROUP, axis=1)
    return w4 * sc_full * np.float32(weight_scale_global)


_H128 = None


def _hadamard128():
    global _H128
    if _H128 is None:
        H = np.array([[1.0]])
        while H.shape[0] < 128:
            H = np.block([[H, H], [H, -H]])
        _H128 = H / np.sqrt(128.0)
    return _H128


_SEL = None


def _fp8_selection():
    """Stratified 3-of-16 sample from every scale group (768 columns)."""
    global _SEL
    if _SEL is None:
        rng = np.random.default_rng(2)
        sel = np.zeros(IN, bool)
        for g in range(IN // GROUP):
            sel[rng.choice(GROUP, KT8 * 128 * GROUP // IN, replace=False)
                + g * GROUP] = True
        _SEL = sel
    return _SEL


def marshal(x, weight_data, weight_scales, weight_scale_global, bias,
            n_cores=NCORES):
    """Host-side marshaling: dequantize + dtype split + device layouts."""
    sel = _fp8_selection()
    H = _hadamard128()
    xf = np.ascontiguousarray(x.reshape(M, IN)).astype(np.float64)
    # bf16 part: [M, XBF] -> [JT,128m,KTB,128p] -> [JT,128p,KTB,128m]
    xb = xf[:, ~sel].astype(BF16)
    xb = xb.reshape(JT, 128, KTB, 128).transpose(0, 3, 2, 1)
    xb = np.ascontiguousarray(xb).reshape(JT * 128, XBF)
    # fp8 part: per-k-tile Hadamard rotation, then pre-scaled e4m3
    xq = xf[:, sel]
    for t in range(KT8):
        s = slice(t * 128, (t + 1) * 128)
        xq[:, s] = xq[:, s] @ H
    x8 = np.clip(xq * X8SCALE, -240.0, 240.0).astype(E4)
    x8 = x8.reshape(JT, 128, KT8, 128).transpose(0, 3, 2, 1)
    x8 = np.ascontiguousarray(x8).reshape(JT * 128, G8, 2, 128)

    W = _dequant_np(weight_data, weight_scales, weight_scale_global).astype(
        np.float64
    )
    # bf16 part: [OUT, XBF] -> [128p, KTB, OUT]
    Wb = W[:, ~sel].astype(BF16)
    Wb = Wb.T.reshape(KTB, 128, OUT).transpose(1, 0, 2)
    # fp8 part: same rotation, scaled down to compensate x8's pre-scale
    Wq = W[:, sel]
    for t in range(KT8):
        s = slice(t * 128, (t + 1) * 128)
        Wq[:, s] = Wq[:, s] @ H
    W8 = np.clip(Wq / X8SCALE, -240.0, 240.0).astype(E4)
    W8 = W8.T.reshape(KT8, 128, OUT).transpose(1, 0, 2)

    bias_f = bias.astype(np.float32)
    in_maps = []
    for c in range(n_cores):
        sl = slice(c * O_C, (c + 1) * O_C)
        in_maps.append(
            {
                "xtb": xb,
                "xt8": x8,
                "wtb": np.ascontiguousarray(Wb[:, :, sl]).reshape(128, WBF),
                "wt8": np.ascontiguousarray(W8[:, :, sl]).reshape(128, G8, 2, O_C),
                "bias": np.ascontiguousarray(bias_f[sl].reshape(1, O_C)),
            }
        )
    return in_maps


_NC_CACHE = {}


def run(x, weight_data, weight_scales, weight_scale_global, bias, trace=False):
    if "mix" not in _NC_CACHE:
        nc = build()
        _split_excess_waits(nc)
        _NC_CACHE["mix"] = nc
    nc = _NC_CACHE["mix"]
    in_maps = marshal(
        np.asarray(x), np.asarray(weight_data), np.asarray(weight_scales),
        np.asarray(weight_scale_global), np.asarray(bias),
    )
    res = run_bass_kernel_spmd(nc, in_maps, list(range(NCORES)), trace=trace)
    y = np.concatenate([res.results[c]["y"] for c in range(NCORES)], axis=1)
    return np.ascontiguousarray(y.reshape(B, S, OUT).astype(np.float32)), res


MODE = "mix-fp8x6-had-v2"


def kernel(x, weight_data, weight_scales, weight_scale_global, bias):
    y, _ = run(x, weight_data, weight_scales, weight_scale_global, bias)
    return y


# revision 35
# speedup vs baseline: 1.1934x; 1.0017x over previous
"""NVFP4 linear layer kernel for Trainium2 (8 NeuronCores) — mixed bf16/fp8.

y = x @ dequant(W)^T + bias, W is FP4(E2M1) with E4M3 group scales and a
global fp32 scale.

Strategy (column-parallel, out-features sharded 8 ways, O_C=512/core):
  - Host: dequantize W to f32 (numpy; bf16-exact since W has a 5-bit
    significand). Contraction split: 768 of 4096 k-columns go to fp8e4
    (TRN E4M3, max 240) as 6 k-tiles, the rest stay bf16. The fp8
    columns are a stratified 3-of-16 sample from every scale group
    (equalizes per-row fp8 energy) and each packed fp8 k-tile is
    rotated by an orthonormal 128-Hadamard on both x and W (exact:
    (xR)(WR)^T = xW^T) to gaussianize the quantization error and kill
    the max-element error tail. fp8 x pre-scaled by 32, fp8 W by 1/32,
    so both halves accumulate at scale 1 into one PSUM chain.
    Measured on HW vs fp64 reference: rel_norm 1.630e-2, rel_max
    1.657e-2 (budget 2e-2).
  - Device: per 128-token j-tile: 26 bf16 matmuls + 3 fp8 DoubleRow
    matmuls (each covering 2 k-tiles at 2x rate) into one PSUM tile,
    bias epilogue on DVE, y DMA out. W resident in SBUF.
  - PE cost: 64 j-tiles x 29 slots x 512 cols = 950272 cycles = 396us
    at 2.4 GHz (vs 437us all-bf16). Measured 430us (vs 470us bf16
    baseline); the ~34us gap is the HBM-bound warmup head (all 8 cores
    stream W+x at once), PE pstate ramp, and fixed preamble/teardown —
    warmup restructure attempts (fp8-first, staggered chains, dummy
    ramp matmuls, finer W pieces) all measured worse.
"""
import os
import sys

for _p in ("/opt/trn_rl_repo", "/root/.axon_site/_ro/trn_rl_repo"):
    if _p not in sys.path and os.path.isdir(_p):
        sys.path.append(_p)

import numpy as np
import ml_dtypes
import concourse.bass as bass
import concourse.mybir as mybir
import concourse.tile as tile
from concourse.alu_op_type import AluOpType
from concourse.bass_utils import run_bass_kernel_spmd

BF16 = ml_dtypes.bfloat16
E4 = ml_dtypes.float8_e4m3  # TRN fp8e4: IEEE-ish E4M3, max normal 240

# Problem shapes (hardcoded per contract).
B, S, IN, OUT = 4, 2048, 4096, 4096
M = B * S                 # 8192 tokens
NCORES = 8
O_C = OUT // NCORES       # 512 out-features per core
KT = IN // 128            # 32 k-tiles of 128 contraction
KTB = 26                  # bf16 k-tiles
KT8 = KT - KTB            # fp8 k-tiles
G8 = KT8 // 2             # fp8 DoubleRow groups (2 k-tiles each)
GROUP = 16
JT = M // 128             # 64 token tiles of 128
XBF = KTB * 128           # bf16 x free elems per partition per j-tile
X8F = KT8 * 128           # fp8 x free elems per partition per j-tile
WBF = KTB * O_C           # bf16 W free elems per partition
W8F = KT8 * O_C           # fp8 W free elems per partition
X8SCALE = 32.0            # fp8 x pre-scale; fp8 W is divided by the same


def _split_excess_waits(nc, maxw=1):
    """walrus CoreV3 accepts at most one sync-wait per instruction; move
    excess waits onto preceding NoOps on the same engine."""
    for f in nc.m.functions:
        for bb in f.blocks:
            new_insts = []
            for inst in bb.instructions:
                si = inst.sync_info
                if si is not None and si.on_wait and len(si.on_wait) > maxw:
                    waits = list(si.on_wait)
                    excess, keep = waits[:-maxw], waits[-maxw:]
                    for i in range(0, len(excess), maxw):
                        new_insts.append(
                            mybir.InstNoOp(
                                name=nc.get_next_instruction_name(),
                                engine=inst.engine,
                                sync_info=mybir.SyncInfo(
                                    on_wait=excess[i : i + maxw], on_update=[]
                                ),
                                bass_nofuse=True,
                            )
                        )
                    si.on_wait = keep
                new_insts.append(inst)
            bb.instructions[:] = new_insts


def build():
    """Per-core SPMD program.

    Inputs (per core):
      xtb  [JT*128, XBF] bf16: row jj*128+p, col t*128+m = x[jj*128+m, t*128+p]
      xt8  [JT*128, X8F] fp8 : row jj*128+p, col t*128+m =
                               32*x[jj*128+m, (KTB+t)*128+p]
      wtb  [128, WBF] bf16: row p, col t*O_C+o = W[o_glob, t*128+p]
      wt8  [128, W8F] fp8 : row p, col t*O_C+o = W[o_glob, (KTB+t)*128+p]/32
      bias [1, O_C] f32
    Output:
      y    [M, O_C] f32
    """
    dt = mybir.dt
    DR = mybir.MatmulPerfMode.DoubleRow
    nc = bass.Bass("TRN2", target_bir_lowering=False, debug=False)
    xtb = nc.dram_tensor("xtb", [JT * 128, XBF], dt.bfloat16, kind="ExternalInput").ap()
    xt8 = nc.dram_tensor(
        "xt8", [JT * 128, G8, 2, 128], dt.float8e4, kind="ExternalInput"
    ).ap()
    wtb = nc.dram_tensor("wtb", [128, WBF], dt.bfloat16, kind="ExternalInput").ap()
    wt8 = nc.dram_tensor(
        "wt8", [128, G8, 2, O_C], dt.float8e4, kind="ExternalInput"
    ).ap()
    bias = nc.dram_tensor("bias", [1, O_C], dt.float32, kind="ExternalInput").ap()
    y = nc.dram_tensor("y", [M, O_C], dt.float32, kind="ExternalOutput").ap()

    with tile.TileContext(nc) as tc:
        with (
            tc.tile_pool(name="persist", bufs=1) as pp,
            tc.tile_pool(name="xtile", bufs=5) as xp,
            tc.tile_pool(name="x8tile", bufs=5) as xp8,
            tc.tile_pool(name="yout", bufs=4) as yp,
            tc.tile_pool(name="psum", bufs=8, space="PSUM") as psp,
        ):
            # ---- persistent tiles: W (resident) + bias ----
            w_b = pp.tile([128, WBF], dt.bfloat16, tag="wtb")
            w_8 = pp.tile([128, G8, 2, O_C], dt.float8e4, tag="wt8")
            # Step order per chain: bf16 k-tiles first (steps 0..KTB-1),
            # then the fp8 DoubleRow groups (steps KTB..NSTEP-1). W is
            # streamed in graded pieces over the two otherwise-idle
            # queues: tiny first pieces so the first matmuls start ASAP.
            engs = [nc.gpsimd, nc.scalar]
            W_PIECES = [(0, 2), (2, 2), (4, 4), (8, 4), (12, 4), (16, 4),
                        (20, 4), (24, 2), (KTB, 2), (KTB + 2, 1)]
            NSTEP = KTB + G8

            def w_dma(eng, s0, sn):
                # contiguous in both wtb (bf16 steps) and wt8 (fp8 steps)
                if s0 < KTB:
                    assert s0 + sn <= KTB
                    eng.dma_start(
                        w_b[:, s0 * O_C : (s0 + sn) * O_C],
                        wtb[:, s0 * O_C : (s0 + sn) * O_C],
                    )
                else:
                    g0 = s0 - KTB
                    eng.dma_start(w_8[:, g0 : g0 + sn], wt8[:, g0 : g0 + sn])

            for h, (s0, sn) in enumerate(W_PIECES):
                w_dma(engs[h % 2], s0, sn)
            # bias last: not needed until the first epilogue, and the
            # 128-way broadcast read is slow — must not block W
            bias_t = pp.tile([128, O_C], dt.float32, tag="bias")
            nc.scalar.dma_start(bias_t[:], bias.broadcast_to([128, O_C]))

            def load_x(jj):
                xc = xp.tile([128, XBF], dt.bfloat16, tag="xc")
                nc.sync.dma_start(xc[:], xtb[jj * 128 : (jj + 1) * 128, :])
                x8c = xp8.tile([128, G8, 2, 128], dt.float8e4, tag="x8c")
                nc.gpsimd.dma_start(x8c[:], xt8[jj * 128 : (jj + 1) * 128])
                return xc, x8c

            def mm(ps, xc, x8c, s):
                # step s: 0..KTB-1 bf16 k-tile, KTB.. fp8 DoubleRow group
                if s < KTB:
                    nc.tensor.matmul(
                        ps[:],
                        xc[:, s * 128 : (s + 1) * 128],
                        w_b[:, s * O_C : (s + 1) * O_C],
                        start=(s == 0),
                        stop=False,
                    )
                else:
                    g = s - KTB
                    nc.tensor.matmul(
                        ps[:],
                        x8c[:, g],
                        w_8[:, g],
                        start=False,
                        stop=(g == G8 - 1),
                        perf_mode=DR,
                    )

            def epilogue(jj, ps):
                yc = yp.tile([128, O_C], dt.float32, tag="yc")
                nc.vector.tensor_tensor(
                    out=yc[:], in0=ps[:], in1=bias_t[:], op=AluOpType.add
                )
                nc.scalar.dma_start(y[jj * 128 : (jj + 1) * 128, :], yc[:])

            # Warmup: first 3 chains interleaved at W-piece granularity so
            # each arriving W piece feeds 3 chains' worth of matmuls while
            # the rest of W is still streaming in. x for these chains is
            # DMA'd in matching pieces, chain-major per piece.
            WARM = 3
            xws = [
                xp.tile([128, XBF], dt.bfloat16, tag="xc", name=f"xw{c}")
                for c in range(WARM)
            ]
            x8ws = [
                xp8.tile([128, G8, 2, 128], dt.float8e4, tag="x8c", name=f"x8w{c}")
                for c in range(WARM)
            ]
            for s0, sn in W_PIECES:
                for c in range(WARM):
                    if s0 < KTB:
                        nc.sync.dma_start(
                            xws[c][:, s0 * 128 : (s0 + sn) * 128],
                            xtb[c * 128 : (c + 1) * 128, s0 * 128 : (s0 + sn) * 128],
                        )
                    else:
                        # fp8 warmup x on gpsimd (idle after its W pieces):
                        # keeps these 9 small DMAs from delaying the main
                        # loop's x tiles behind them on the sync queue
                        g0 = s0 - KTB
                        nc.gpsimd.dma_start(
                            x8ws[c][:, g0 : g0 + sn],
                            xt8[c * 128 : (c + 1) * 128, g0 : g0 + sn],
                        )
            psw = [
                psp.tile([128, O_C], dt.float32, tag="ps", name=f"psw{c}")
                for c in range(WARM)
            ]
            for s0, sn in W_PIECES:
                for c in range(WARM):
                    for s in range(s0, s0 + sn):
                        mm(psw[c], xws[c], x8ws[c], s)
            for c in range(WARM):
                epilogue(c, psw[c])

            for jj in range(WARM, JT):
                xc, x8c = load_x(jj)
                ps = psp.tile([128, O_C], dt.float32, tag="ps")
                for s in range(NSTEP):
                    mm(ps, xc, x8c, s)
                epilogue(jj, ps)

    return nc


def _dequant_np(weight_data, weight_scales, weight_scale_global):
    """numpy port of reference.dequantize_fp4 -> f32 [OUT, IN]."""
    fp4_lut = np.array(
        [0.0, 0.5, 1.0, 1.5, 2.0, 3.0, 4.0, 6.0,
         -0.0, -0.5, -1.0, -1.5, -2.0, -3.0, -4.0, -6.0], dtype=np.float32)
    b = np.arange(256)
    s = np.where((b >> 7) & 1, -1.0, 1.0)
    e = (b >> 3) & 0xF
    m = (b & 7).astype(np.float64)
    normal = s * np.exp2(e - 7.0) * (1.0 + m / 8.0)
    subnormal = s * np.exp2(-6.0) * (m / 8.0)
    e4m3_lut = np.where(e == 0, subnormal, normal).astype(np.float32)

    wd = weight_data.astype(np.int64)
    o_dim = wd.shape[0]
    lo = wd & 0xF
    hi = (wd >> 4) & 0xF
    nib = np.stack([lo, hi], axis=-1).reshape(o_dim, -1)
    w4 = fp4_lut[nib]
    sc = e4m3_lut[weight_scales.astype(np.int64)]
    sc_full = np.repeat(sc, GROUP, axis=1)
    return w4 * sc_full * np.float32(weight_scale_global)


_H128 = None


def _hadamard128():
    global _H128
    if _H128 is None:
        H = np.array([[1.0]])
        while H.shape[0] < 128:
            H = np.block([[H, H], [H, -H]])
        _H128 = H / np.sqrt(128.0)
    return _H128


_SEL = None


def _fp8_selection():
    """Stratified 3-of-16 sample from every scale group (768 columns)."""
    global _SEL
    if _SEL is None:
        rng = np.random.default_rng(2)
        sel = np.zeros(IN, bool)
        for g in range(IN // GROUP):
            sel[rng.choice(GROUP, KT8 * 128 * GROUP // IN, replace=False)
                + g * GROUP] = True
        _SEL = sel
    return _SEL


def marshal(x, weight_data, weight_scales, weight_scale_global, bias,
            n_cores=NCORES):
    """Host-side marshaling: dequantize + dtype split + device layouts."""
    sel = _fp8_selection()
    H = _hadamard128()
    xf = np.ascontiguousarray(x.reshape(M, IN)).astype(np.float64)
    # bf16 part: [M, XBF] -> [JT,128m,KTB,128p] -> [JT,128p,KTB,128m]
    xb = xf[:, ~sel].astype(BF16)
    xb = xb.reshape(JT, 128, KTB, 128).transpose(0, 3, 2, 1)
    xb = np.ascontiguousarray(xb).reshape(JT * 128, XBF)
    # fp8 part: per-k-tile Hadamard rotation, then pre-scaled e4m3
    xq = xf[:, sel]
    for t in range(KT8):
        s = slice(t * 128, (t + 1) * 128)
        xq[:, s] = xq[:, s] @ H
    x8 = np.clip(xq * X8SCALE, -240.0, 240.0).astype(E4)
    x8 = x8.reshape(JT, 128, KT8, 128).transpose(0, 3, 2, 1)
    x8 = np.ascontiguousarray(x8).reshape(JT * 128, G8, 2, 128)

    W = _dequant_np(weight_data, weight_scales, weight_scale_global).astype(
        np.float64
    )
    # bf16 part: [OUT, XBF] -> [128p, KTB, OUT]
    Wb = W[:, ~sel].astype(BF16)
    Wb = Wb.T.reshape(KTB, 128, OUT).transpose(1, 0, 2)
    # fp8 part: same rotation, scaled down to compensate x8's pre-scale
    Wq = W[:, sel]
    for t in range(KT8):
        s = slice(t * 128, (t + 1) * 128)
        Wq[:, s] = Wq[:, s] @ H
    W8 = np.clip(Wq / X8SCALE, -240.0, 240.0).astype(E4)
    W8 = W8.T.reshape(KT8, 128, OUT).transpose(1, 0, 2)

    bias_f = bias.astype(np.float32)
    in_maps = []
    for c in range(n_cores):
        sl = slice(c * O_C, (c + 1) * O_C)
        in_maps.append(
            {
                "xtb": xb,
                "xt8": x8,
                "wtb": np.ascontiguousarray(Wb[:, :, sl]).reshape(128, WBF),
                "wt8": np.ascontiguousarray(W8[:, :, sl]).reshape(128, G8, 2, O_C),
                "bias": np.ascontiguousarray(bias_f[sl].reshape(1, O_C)),
            }
        )
    return in_maps


_NC_CACHE = {}


def run(x, weight_data, weight_scales, weight_scale_global, bias, trace=False):
    if "mix" not in _NC_CACHE:
        nc = build()
        _split_excess_waits(nc)
        _NC_CACHE["mix"] = nc
    nc = _NC_CACHE["mix"]
    in_maps = marshal(
        np.asarray(x), np.asarray(weight_data), np.asarray(weight_scales),
        np.asarray(weight_scale_global), np.asarray(bias),
    )
    res = run_bass_kernel_spmd(nc, in_maps, list(range(NCORES)), trace=trace)
    y = np.concatenate([res.results[c]["y"] for c in range(NCORES)], axis=1)
    return np.ascontiguousarray(y.reshape(B, S, OUT).astype(np.float32)), res


MODE = "mix-fp8x6-had-v2"


def kernel(x, weight_data, weight_scales, weight_scale_global, bias):
    y, _ = run(x, weight_data, weight_scales, weight_scale_global, bias)
    return y
